# revision 29
# baseline (speedup 1.0000x reference)
"""EuclideanFastAttention Trainium2 kernel (Gram-matrix / A-form version).

Full inputs -> shard graphs across 8 NeuronCores (1 graph/core) -> per-core
Bass/Tile kernel (Euclidean RoPE + linear attention over Lebedev quadrature)
-> gather full output.

Math (per core = per graph, M=256 nodes, D=256 features, 7 antipodal pairs):
  out = sum_g w_g Q_g (K_g^T V)  ==  A @ V   with  A = sum_g w_g Q_g Q_g^T
(q and k are both the masked rotated x: masking k zeroes padded KV rows,
and masking q zeroes the same output rows the reference masks at the end,
so the node mask is folded into x once on the host). Per antipodal pair t
the cross terms cancel, and because a Gram matrix is invariant under the
signed pair-swap inside each (s,j) block ((S.xs)(S.xs)^T == (S.x)(S.x)^T),
the swapped operand xs disappears:
  A += w_t [ (c_t.x)(c_t.x)^T + (s_t.x)(s_t.x)^T ].
The 14-point Lebedev grid has exactly two distinct weights (octahedron /
cube vertices), so terms are grouped by weight: A = w0*A0 + w1*A1 with
UNWEIGHTED accumulators; w_g is applied exactly (f32) in the A_g escape.

Device pipeline per pair term t:
  k_bf = {cos_t, sin_t} * x         (DVE muls, bf16)
  qt   = k_bf^T                     (8 PE transposes -> PSUM bf16)
  qt8  = fp8(qt)                    (ACT/DVE escape)
  A_g += qt8^T qt8                  (fp8 DoubleRow matmuls, K=256/instr)
Group 0 (3 terms) finishes early: its escape + A0@V overlap group 1's
terms. out lands in PSUM f32, is escaped to bf16 and DMA'd out; the host
un-permutes and casts back to f32.

Host prep folds: node mask into x, d-permutation (s,f)->(s,r,j), bf16
conversion, frequencies/(2*pi), pair-weight grouping.

Self-contained: hardcodes the problem geometry (N=2048, B=8, P=1, S=4, F=64,
G=14, J=32) but derives everything it can from the input arrays at runtime.
"""
import sys

sys.path.insert(0, "/opt/trn_rl_repo")

import ml_dtypes
import numpy as np

import concourse.bacc as bacc
import concourse.bass as bass
import concourse.mybir as mybir
import concourse.tile as tile
from concourse import masks
from concourse.bass_utils import run_bass_kernel_spmd

F32 = mybir.dt.float32
BF16 = mybir.dt.bfloat16
E4 = mybir.dt.float8e4
ACTF = mybir.ActivationFunctionType
ALU = mybir.AluOpType
DR = mybir.MatmulPerfMode.DoubleRow

PI = float(np.pi)
TWO_PI = float(2.0 * np.pi)
MAGIC = float(1.5 * 2.0**23)  # fp32 round-to-nearest-int magic constant

N_CORES = 8
NT = 7        # antipodal pair terms
NG0 = 3       # terms in weight-group 0 (octahedron pairs)
J = 32        # RoPE frequency pairs
D = 256       # p*s*f
M = 256       # nodes per graph
NK = 2        # 128-node chunks

# aux column map
C_POST = 0            # posT [3, 256]
C_UT = C_POST + M     # uT [3, 7]
C_FRQ = C_UT + NT     # freq/(2pi) [32] (row 0)
C_W = C_FRQ + J       # group pair-weights [2] (row 0)
W_AUX = C_W + 2


def _ap(t_ap, off, dims):
    return bass.AP(tensor=t_ap.tensor, offset=t_ap.offset + off,
                   ap=[list(t_ap.ap[0])] + [list(d) for d in dims])


def _build_program():
    """SPMD per-core program. DRAM params:
    x   [128, 512] bf16: masked x, d-order (s,r,j), node = c*128+p
    aux [128, W_AUX] f32 (rows 0:3 used)
    out [128, 512] bf16: same layout as x
    """
    nc = bacc.Bacc()
    X = nc.declare_dram_parameter("x", [128, NK * D], BF16, isOutput=False)
    AUX = nc.declare_dram_parameter("aux", [128, W_AUX], F32, isOutput=False)
    OUT = nc.declare_dram_parameter("out", [128, NK * D], BF16, isOutput=True)

    with tile.TileContext(nc) as tc:
        with (
            tc.tile_pool(name="const", bufs=1) as cp,
            tc.tile_pool(name="kbf", bufs=6) as kbfp,
            tc.tile_pool(name="qt8", bufs=4) as qtp,
            tc.tile_pool(name="qtb", bufs=3) as qtbp,
            tc.tile_pool(name="ang", bufs=6) as angp,
            tc.tile_pool(name="scb", bufs=5) as scp,
            tc.tile_pool(name="qtps", bufs=4, space="PSUM") as qtps,
            tc.tile_pool(name="warmp", bufs=1, space="PSUM") as warmp,
            tc.tile_pool(name="accps", bufs=1, space="PSUM") as accps,
        ):
            # ---------------- loads (both on the SP queue) ----------------
            aux_sb = cp.tile([128, W_AUX], F32)
            nc.sync.dma_start(out=aux_sb[0:3, :], in_=AUX[0:3, :])
            x_sb = cp.tile([128, NK * D], BF16)
            nc.sync.dma_start(out=x_sb, in_=X[:, :])

            zero_col = cp.tile([128, 1], F32)
            nc.vector.memset(zero_col, 0.0)

            identf = cp.tile([128, 128], F32)
            masks.make_identity(nc, identf[:])
            identb = cp.tile([128, 128], BF16)
            nc.vector.tensor_copy(identb, identf)

            # ---------------- dpt / broadcasts ----------------
            # dpt[p, (c, t)] = u_t . r_(c*128+p)
            out_ps = accps.tile([128, 2 * D], F32, tag="out", name="out_ps")
            dpt_ps = out_ps[:, 0:2 * NT]
            for c in range(NK):
                nc.tensor.matmul(
                    dpt_ps[:, c * NT:(c + 1) * NT],
                    aux_sb[0:3, C_POST + c * 128:C_POST + (c + 1) * 128],
                    aux_sb[0:3, C_UT:C_UT + NT],
                    start=True, stop=True,
                )
            dpt = cp.tile([128, 2 * NT], F32)
            nc.vector.tensor_copy(dpt, dpt_ps)

            # freq/(2pi) broadcast [128, J] (host pre-divided)
            frq_bc = cp.tile([128, J], F32)
            nc.gpsimd.partition_broadcast(frq_bc, aux_sb[0:1, C_FRQ:C_FRQ + J])

            # ---------------- angle chains ----------------
            # ang[p,(c,j)] = dpt[p,c,t] * f[j]/2pi; cos via +0.25 shift; range
            # reduce with MAGIC; sc = Sin(2pi d) -> bf16 [p,(sc2,c2,j32)].
            def emit_angle_pair(ts, fast=False):
                n = len(ts)
                eng = nc.vector if fast else nc.gpsimd
                w_t = angp.tile([128, n * 2 * J], F32, tag="w", name="w_t")
                for i, t in enumerate(ts):
                    eng.tensor_mul(
                        w_t[:, i * 2 * J:(i + 1) * 2 * J].rearrange(
                            "p (c j) -> p c j", c=2),
                        _ap(dpt_ps if fast else dpt[:], t, [[NT, 2], [0, J]]),
                        _ap(frq_bc[:], 0, [[0, 2], [1, J]]),
                    )
                kb_t = angp.tile([128, n * 4 * J], F32, tag="kb", name="kb_t")
                tcos = angp.tile([128, n * 2 * J], F32, tag="tc", name="tcos")
                eng.tensor_scalar(tcos, w_t, 0.25, MAGIC, ALU.add, ALU.add)
                eng.tensor_scalar(
                    _ap(kb_t[:], 0, [[4 * J, n], [1, 2 * J]]),
                    tcos[:].rearrange("p (t cj) -> p t cj", t=n),
                    MAGIC, 0.25, ALU.subtract, ALU.subtract)
                eng.tensor_scalar(
                    _ap(kb_t[:], 2 * J, [[4 * J, n], [1, 2 * J]]),
                    w_t[:].rearrange("p (t cj) -> p t cj", t=n),
                    MAGIC, MAGIC, ALU.add, ALU.subtract)
                d_t = angp.tile([128, n * 4 * J], F32, tag="d", name="d_t")
                eng.tensor_sub(
                    d_t[:].rearrange("p (t sc cj) -> p t sc cj", t=n, sc=2),
                    _ap(w_t[:], 0, [[2 * J, n], [0, 2], [1, 2 * J]]),
                    kb_t[:].rearrange("p (t sc cj) -> p t sc cj", t=n, sc=2),
                )
                sc_t = scp.tile([128, n * 4 * J], BF16, tag="sc", name="sc_t")
                nc.scalar.activation(sc_t, d_t, ACTF.Sin, bias=zero_col[:, 0:1],
                                     scale=TWO_PI)
                return {t: sc_t[:, i * 4 * J:(i + 1) * 4 * J]
                        for i, t in enumerate(ts)}

            # ---------------- per-term pipeline ----------------
            A_g0 = accps.tile([128, 2 * D], F32, tag="Ag0", name="A_g0")
            A_g1 = accps.tile([128, 2 * D], F32, tag="Ag1", name="A_g1")

            def emit_muls(t, sc_ap):
                k_bf = kbfp.tile([128, 2 * NK * D], BF16, tag="kbf",
                                 name="k_bf")
                for part in range(2):
                    nc.vector.tensor_mul(
                        k_bf[:, part * 512:part * 512 + 512].rearrange(
                            "p (c st j) -> p c st j", c=2, st=8),
                        x_sb[:].rearrange("p (c st j) -> p c st j", c=2, st=8),
                        _ap(sc_ap, part * 2 * J, [[J, 2], [0, 8], [1, J]]),
                    )
                return k_bf

            def emit_qt(t, k_bf, full_act=False):
                # transposes: qt_ps[p=d%128, (part, dc, c, m)]
                qt_ps = qtps.tile([128, 1024], BF16, tag="qt", name="qt_ps")
                for part in range(2):
                    for c in range(NK):
                        for dc in range(2):
                            nc.tensor.matmul(
                                qt_ps[:, part * 512 + dc * 256 + c * 128:
                                      part * 512 + dc * 256 + c * 128 + 128],
                                k_bf[:, part * 512 + c * 256 + dc * 128:
                                     part * 512 + c * 256 + dc * 128 + 128],
                                identb,
                                is_transpose=True,
                            )
                # escape -> fp8 (no scale; group weight applied at A escape)
                qt_sb = qtp.tile([128, 1024], E4, tag="qtsb", name="qt_sb")
                if full_act:
                    nc.scalar.activation(qt_sb, qt_ps, ACTF.Copy)
                else:
                    nc.scalar.activation(qt_sb[:, 0:768], qt_ps[:, 0:768],
                                         ACTF.Copy)
                    nc.vector.tensor_copy(qt_sb[:, 768:1024],
                                          qt_ps[:, 768:1024])
                return ("f8", qt_sb)

            def emit_qt_dma(t, k_bf):
                # DMA xbar transpose, no compute engine: k_bf[m, (part,c,dh,
                # dl)] -> qt_bf[dl, (part, c, dh, m)] (g = (part,c,dh) blocks)
                qt_bf = qtbp.tile([128, 1024], BF16, tag="qtbf", name="qt_bf")
                nc.sync.dma_start_transpose(
                    qt_bf[:].rearrange("p (g m) -> p g m", g=8),
                    k_bf[:],
                )
                return ("bf", qt_bf)

            def emit_A(t, qt):
                kind, qt_sb = qt
                acc = A_g0 if t < NG0 else A_g1
                first = t in (0, NG0)
                last = t in (NG0 - 1, NT - 1)
                if kind == "f8":
                    # qt_sb fp8 [128, (part, dc, c*128+m)]: DR, K=256/instr
                    for part in range(2):
                        for mc in range(2):
                            nc.tensor.matmul(
                                acc[:, mc * 256:mc * 256 + 256],
                                _ap(qt_sb[:], part * 512 + mc * 128,
                                    [[256, 2], [1, 128]]),
                                _ap(qt_sb[:], part * 512,
                                    [[256, 2], [1, 256]]),
                                start=(first and part == 0 and mc == 0),
                                stop=(last and part == 1 and mc == 1),
                                perf_mode=DR,
                            )
                    return
                # qt_sb bf16 [128, (part, c, dh, m)]: K=128/instr
                n8 = 0
                for part in range(2):
                    for mc in range(2):
                        for dh in range(2):
                            n8 += 1
                            nc.tensor.matmul(
                                acc[:, mc * 256:mc * 256 + 256],
                                qt_sb[:, part * 512 + mc * 256 + dh * 128:
                                      part * 512 + mc * 256 + dh * 128 + 128],
                                _ap(qt_sb[:], part * 512 + dh * 128,
                                    [[256, 2], [1, 128]]),
                                start=(first and n8 == 1),
                                stop=(last and n8 == 8),
                            )

            def emit_Aesc(a_ps, a_sb, w_col, eng):
                # A_sb[p, (mc, m')] = w_g * A[mc*128+p, m'] (exact f32 w)
                if eng == "dve":
                    nc.vector.tensor_scalar_mul(a_sb, a_ps, w_col)
                else:
                    nc.scalar.activation(a_sb, a_ps, ACTF.Copy, scale=w_col)

            def emit_AVmm(a_sb, start, stop):
                # symmetric A: the same tile serves as lhsT for both chunks
                for oc in range(2):
                    for kc in range(2):
                        nc.tensor.matmul(
                            out_ps[:, oc * 256:oc * 256 + 256],
                            a_sb[:, kc * 256 + oc * 128:
                                 kc * 256 + oc * 128 + 128],
                            x_sb[:, kc * 256:(kc + 1) * 256],
                            start=(start and oc == 0 and kc == 0),
                            stop=(stop and oc == 1 and kc == 1),
                        )

            DMA_TERMS = (0, 1)

            sc_tiles = {}
            sc_tiles.update(emit_angle_pair([0, 1], fast=True))
            sc_tiles.update(emit_angle_pair([2], fast=True))
            sc_tiles.update(emit_angle_pair([3, 4], fast=False))

            A_sb0 = cp.tile([128, 2 * D], BF16)
            A_sb1 = cp.tile([128, 2 * D], BF16)

            qts = {}
            for t in range(NT):
                k_bf = emit_muls(t, sc_tiles.pop(t))
                if t == 0:
                    sc_tiles.update(emit_angle_pair([5, 6], fast=False))
                # PE keep-alive: one transpose per k_bf keeps the p-state
                # ramp warm across the window where PE waits for dma-qt.
                warm_ps = warmp.tile([128, 128], BF16, tag="warm",
                                     name="warm_ps")
                nc.tensor.matmul(warm_ps[:, 0:128], k_bf[:, 0:128], identb,
                                 is_transpose=True)
                if t in DMA_TERMS:
                    qts[t] = emit_qt_dma(t, k_bf)
                else:
                    qts[t] = emit_qt(t, k_bf, full_act=(t < 4))
            # group pair-weights [128, 2] (needed only by the A escapes)
            w_bc = cp.tile([128, 2], F32)
            nc.gpsimd.partition_broadcast(w_bc, aux_sb[0:1, C_W:C_W + 2])
            for t in range(NT):
                emit_A(t, qts.pop(t))
            emit_Aesc(A_g0, A_sb0, w_bc[:, 0:1], "dve")
            emit_Aesc(A_g1, A_sb1, w_bc[:, 1:2], "act")
            emit_AVmm(A_sb0, start=True, stop=False)
            emit_AVmm(A_sb1, start=False, stop=True)

            # ---------------- tail: escape + store (per chunk) ----------
            out_sb = cp.tile([128, NK * D], BF16)
            nc.scalar.activation(out_sb[:, 0:256], out_ps[:, 0:256], ACTF.Copy)
            nc.vector.tensor_copy(out_sb[:, 256:512], out_ps[:, 256:512])
            nc.sync.dma_start(out=OUT[:, 0:256], in_=out_sb[:, 0:256])
            nc.scalar.dma_start(out=OUT[:, 256:512], in_=out_sb[:, 256:512])

    nc.finalize()
    return nc


_PROGRAM_CACHE = {}


def _get_program():
    if "p" not in _PROGRAM_CACHE:
        _PROGRAM_CACHE["p"] = _build_program()
    return _PROGRAM_CACHE["p"]


def _find_pairs(grid_u, grid_w):
    """Antipodal pairs with equal weights; assert full pairing."""
    G = grid_u.shape[0]
    used = [False] * G
    pairs = []
    for i in range(G):
        if used[i]:
            continue
        partner = -1
        for j in range(i + 1, G):
            if used[j]:
                continue
            if (np.allclose(grid_u[j], -grid_u[i], rtol=1e-6, atol=1e-7)
                    and abs(float(grid_w[j]) - float(grid_w[i])) <= 1e-7):
                partner = j
                break
        used[i] = True
        assert partner >= 0, "unpaired grid direction"
        used[partner] = True
        pairs.append((i, partner))
    return pairs


def _prepare(inputs, positions, batch_segments, graph_mask, frequencies, grid_u,
             grid_w):
    n, p, s, f = inputs.shape
    d = p * s * f
    b = graph_mask.shape[0]
    G = grid_u.shape[0]
    Jn = frequencies.shape[0]
    assert (n, d, b, G, Jn) == (2048, 256, 8, 14, 32), (n, d, b, G, Jn)

    x = np.asarray(inputs, np.float32).reshape(n, d)
    pos = np.asarray(positions, np.float32)
    seg = np.asarray(batch_segments)
    gmask = np.asarray(graph_mask)
    gu = np.asarray(grid_u, np.float32)
    gw = np.asarray(grid_w, np.float32)

    idxs = [np.nonzero(seg == c)[0] for c in range(b)]
    assert max(len(ix) for ix in idxs) <= M

    pairs = _find_pairs(gu, gw)
    assert len(pairs) == NT
    # group pairs by weight: exactly two distinct values, sizes (NG0, NT-NG0)
    pw = np.array([gw[i] + gw[j] for i, j in pairs], np.float32)
    vals = np.unique(pw)
    assert len(vals) == 2, vals
    g0 = [k for k in range(NT) if pw[k] == vals[0]]
    g1 = [k for k in range(NT) if pw[k] == vals[1]]
    if len(g0) != NG0:
        g0, g1 = g1, g0
    assert len(g0) == NG0 and len(g1) == NT - NG0, (g0, g1)
    order = g0 + g1
    pairs = [pairs[k] for k in order]
    wgrp = np.array([pw[order[0]], pw[order[NG0]]], np.float32)
    reps = [i for i, _ in pairs]

    in_maps = []
    for c in range(b):
        ix = idxs[c]
        pad = np.zeros(M, np.int64)
        pad[:len(ix)] = ix
        mask = np.zeros(M, np.float32)
        mask[:len(ix)] = gmask[seg[ix]].astype(np.float32)

        xm = x[pad] * mask[:, None]
        # d-permute (s, f=2j+r) -> (s, r, j)
        xp = xm.reshape(M, 4, J, 2).transpose(0, 1, 3, 2).reshape(M, d)
        x_prep = np.ascontiguousarray(
            xp.reshape(NK, 128, d).transpose(1, 0, 2).reshape(128, NK * d))
        aux = np.zeros((128, W_AUX), np.float32)
        aux[0:3, C_POST:C_POST + M] = pos[pad].T
        aux[0:3, C_UT:C_UT + NT] = gu[reps].T
        aux[0, C_FRQ:C_FRQ + J] = (np.asarray(frequencies, np.float32)
                                   / np.float32(TWO_PI))
        aux[0, C_W:C_W + 2] = wgrp
        in_maps.append(dict(x=x_prep.astype(ml_dtypes.bfloat16), aux=aux))

    meta = dict(n=n, p=p, s=s, f=f, d=d, b=b, idxs=idxs, pairs=pairs)
    return in_maps, meta


def _gather(results, meta, dtype):
    n, d = meta["n"], meta["d"]
    out = np.zeros((n, d), np.float32)
    for c, ix in enumerate(meta["idxs"]):
        o = np.asarray(results[c]["out"]).astype(np.float32)
        o_nodes = o.reshape(128, NK, d).transpose(1, 0, 2).reshape(M, d)
        # un-permute (s, r, j) -> (s, f=2j+r)
        o_un = o_nodes.reshape(M, 4, 2, J).transpose(0, 1, 3, 2).reshape(M, d)
        out[ix] = o_un[:len(ix)]
    return out.reshape(n, meta["p"], meta["s"], meta["f"]).astype(dtype)


def _run(inputs, positions, batch_segments, graph_mask, frequencies, grid_u,
         grid_w, trace=False):
    in_maps, meta = _prepare(inputs, positions, batch_segments, graph_mask,
                             frequencies, grid_u, grid_w)
    nc = _get_program()
    res = run_bass_kernel_spmd(
        nc, in_maps, core_ids=list(range(N_CORES)), trace=trace
    )
    out = _gather(res.results, meta, np.asarray(inputs).dtype)
    return out, res


def kernel(inputs, positions, batch_segments, graph_mask, frequencies, grid_u,
           grid_w):
    out, _ = _run(inputs, positions, batch_segments, graph_mask, frequencies,
                  grid_u, grid_w)
    return out


# revision 30
# speedup vs baseline: 1.0265x; 1.0265x over previous
"""EuclideanFastAttention Trainium2 kernel (Gram-matrix / A-form version).

Full inputs -> shard graphs across 8 NeuronCores (1 graph/core) -> per-core
Bass/Tile kernel (Euclidean RoPE + linear attention over Lebedev quadrature)
-> gather full output.

Math (per core = per graph, M=256 nodes, D=256 features, 7 antipodal pairs):
  out = sum_g w_g Q_g (K_g^T V)  ==  A @ V   with  A = sum_g w_g Q_g Q_g^T
(q and k are both the masked rotated x: masking k zeroes padded KV rows,
and masking q zeroes the same output rows the reference masks at the end,
so the node mask is folded into x once on the host). Per antipodal pair t
the cross terms cancel, and because a Gram matrix is invariant under the
signed pair-swap inside each (s,j) block ((S.xs)(S.xs)^T == (S.x)(S.x)^T),
the swapped operand xs disappears:
  A += w_t [ (c_t.x)(c_t.x)^T + (s_t.x)(s_t.x)^T ].
The 14-point Lebedev grid has exactly two distinct weights (octahedron /
cube vertices), so terms are grouped by weight: A = w0*A0 + w1*A1 with
UNWEIGHTED accumulators; w_g is applied exactly (f32) in the A_g escape.

Device pipeline per pair term t:
  k_bf = {cos_t, sin_t} * x         (DVE muls, bf16)
  qt   = k_bf^T                     (8 PE transposes -> PSUM bf16)
  qt8  = fp8(qt)                    (ACT/DVE escape)
  A_g += qt8^T qt8                  (fp8 DoubleRow matmuls, K=256/instr)
Group 0 (3 terms) finishes early: its escape + A0@V overlap group 1's
terms. out lands in PSUM f32, is escaped to bf16 and DMA'd out; the host
un-permutes and casts back to f32.

Host prep folds: node mask into x, d-permutation (s,f)->(s,r,j), bf16
conversion, frequencies/(2*pi), pair-weight grouping.

Self-contained: hardcodes the problem geometry (N=2048, B=8, P=1, S=4, F=64,
G=14, J=32) but derives everything it can from the input arrays at runtime.
"""
import sys

sys.path.insert(0, "/opt/trn_rl_repo")

import ml_dtypes
import numpy as np

import concourse.bacc as bacc
import concourse.bass as bass
import concourse.mybir as mybir
import concourse.tile as tile
from concourse import masks
from concourse.bass_utils import run_bass_kernel_spmd

F32 = mybir.dt.float32
BF16 = mybir.dt.bfloat16
E4 = mybir.dt.float8e4
ACTF = mybir.ActivationFunctionType
ALU = mybir.AluOpType
DR = mybir.MatmulPerfMode.DoubleRow

PI = float(np.pi)
TWO_PI = float(2.0 * np.pi)
MAGIC = float(1.5 * 2.0**23)  # fp32 round-to-nearest-int magic constant

N_CORES = 8
NT = 7        # antipodal pair terms
NG0 = 3       # terms in weight-group 0 (octahedron pairs)
J = 32        # RoPE frequency pairs
D = 256       # p*s*f
M = 256       # nodes per graph
NK = 2        # 128-node chunks

# aux column map
C_POST = 0            # posT [3, 256]
C_UT = C_POST + M     # uT [3, 7]
C_FRQ = C_UT + NT     # freq/(2pi) [32] (row 0)
C_W = C_FRQ + J       # group pair-weights [2] (row 0)
W_AUX = C_W + 2


def _ap(t_ap, off, dims):
    return bass.AP(tensor=t_ap.tensor, offset=t_ap.offset + off,
                   ap=[list(t_ap.ap[0])] + [list(d) for d in dims])


def _build_program():
    """SPMD per-core program. DRAM params:
    x   [128, 512] bf16: masked x, d-order (s,r,j), node = c*128+p
    aux [128, W_AUX] f32 (rows 0:3 used)
    out [128, 512] bf16: same layout as x
    """
    nc = bacc.Bacc()
    X = nc.declare_dram_parameter("x", [128, NK * D], BF16, isOutput=False)
    AUX = nc.declare_dram_parameter("aux", [128, W_AUX], F32, isOutput=False)
    OUT = nc.declare_dram_parameter("out", [128, NK * D], BF16, isOutput=True)

    with tile.TileContext(nc) as tc:
        with (
            tc.tile_pool(name="const", bufs=1) as cp,
            tc.tile_pool(name="kbf", bufs=6) as kbfp,
            tc.tile_pool(name="qt8", bufs=4) as qtp,
            tc.tile_pool(name="qtb", bufs=3) as qtbp,
            tc.tile_pool(name="ang", bufs=6) as angp,
            tc.tile_pool(name="scb", bufs=5) as scp,
            tc.tile_pool(name="qtps", bufs=4, space="PSUM") as qtps,
            tc.tile_pool(name="warmp", bufs=1, space="PSUM") as warmp,
            tc.tile_pool(name="accps", bufs=1, space="PSUM") as accps,
        ):
            # ---------------- loads (both on the SP queue) ----------------
            aux_sb = cp.tile([128, W_AUX], F32)
            nc.sync.dma_start(out=aux_sb[0:3, :], in_=AUX[0:3, :])
            x_sb = cp.tile([128, NK * D], BF16)
            nc.sync.dma_start(out=x_sb, in_=X[:, :])

            zero_col = cp.tile([128, 1], F32)
            nc.vector.memset(zero_col, 0.0)

            identf = cp.tile([128, 128], F32)
            masks.make_identity(nc, identf[:])
            identb = cp.tile([128, 128], BF16)
            nc.vector.tensor_copy(identb, identf)

            # ---------------- dpt / broadcasts ----------------
            # dpt[p, (c, t)] = u_t . r_(c*128+p)
            out_ps = accps.tile([128, 2 * D], F32, tag="out", name="out_ps")
            dpt_ps = out_ps[:, 0:2 * NT]
            for c in range(NK):
                nc.tensor.matmul(
                    dpt_ps[:, c * NT:(c + 1) * NT],
                    aux_sb[0:3, C_POST + c * 128:C_POST + (c + 1) * 128],
                    aux_sb[0:3, C_UT:C_UT + NT],
                    start=True, stop=True,
                )
            dpt = cp.tile([128, 2 * NT], F32)
            nc.vector.tensor_copy(dpt, dpt_ps)

            # freq/(2pi) broadcast [128, J] (host pre-divided)
            frq_bc = cp.tile([128, J], F32)
            nc.gpsimd.partition_broadcast(frq_bc, aux_sb[0:1, C_FRQ:C_FRQ + J])

            # ---------------- angle chains ----------------
            # ang[p,(c,j)] = dpt[p,c,t] * f[j]/2pi; cos via +0.25 shift; range
            # reduce with MAGIC; sc = Sin(2pi d) -> bf16 [p,(sc2,c2,j32)].
            def emit_angle_pair(ts, fast=False):
                n = len(ts)
                eng = nc.vector if fast else nc.gpsimd
                w_t = angp.tile([128, n * 2 * J], F32, tag="w", name="w_t")
                for i, t in enumerate(ts):
                    eng.tensor_mul(
                        w_t[:, i * 2 * J:(i + 1) * 2 * J].rearrange(
                            "p (c j) -> p c j", c=2),
                        _ap(dpt_ps if fast else dpt[:], t, [[NT, 2], [0, J]]),
                        _ap(frq_bc[:], 0, [[0, 2], [1, J]]),
                    )
                kb_t = angp.tile([128, n * 4 * J], F32, tag="kb", name="kb_t")
                tcos = angp.tile([128, n * 2 * J], F32, tag="tc", name="tcos")
                eng.tensor_scalar(tcos, w_t, 0.25, MAGIC, ALU.add, ALU.add)
                eng.tensor_scalar(
                    _ap(kb_t[:], 0, [[4 * J, n], [1, 2 * J]]),
                    tcos[:].rearrange("p (t cj) -> p t cj", t=n),
                    MAGIC, 0.25, ALU.subtract, ALU.subtract)
                eng.tensor_scalar(
                    _ap(kb_t[:], 2 * J, [[4 * J, n], [1, 2 * J]]),
                    w_t[:].rearrange("p (t cj) -> p t cj", t=n),
                    MAGIC, MAGIC, ALU.add, ALU.subtract)
                d_t = angp.tile([128, n * 4 * J], F32, tag="d", name="d_t")
                eng.tensor_sub(
                    d_t[:].rearrange("p (t sc cj) -> p t sc cj", t=n, sc=2),
                    _ap(w_t[:], 0, [[2 * J, n], [0, 2], [1, 2 * J]]),
                    kb_t[:].rearrange("p (t sc cj) -> p t sc cj", t=n, sc=2),
                )
                sc_t = scp.tile([128, n * 4 * J], BF16, tag="sc", name="sc_t")
                nc.scalar.activation(sc_t, d_t, ACTF.Sin, bias=zero_col[:, 0:1],
                                     scale=TWO_PI)
                return {t: sc_t[:, i * 4 * J:(i + 1) * 4 * J]
                        for i, t in enumerate(ts)}

            # ---------------- per-term pipeline ----------------
            A_g0 = accps.tile([128, 2 * D], F32, tag="Ag0", name="A_g0")
            A_g1 = accps.tile([128, 2 * D], F32, tag="Ag1", name="A_g1")

            def emit_muls(t, sc_ap):
                k_bf = kbfp.tile([128, 2 * NK * D], BF16, tag="kbf",
                                 name="k_bf")
                for part in range(2):
                    nc.vector.tensor_mul(
                        k_bf[:, part * 512:part * 512 + 512].rearrange(
                            "p (c st j) -> p c st j", c=2, st=8),
                        x_sb[:].rearrange("p (c st j) -> p c st j", c=2, st=8),
                        _ap(sc_ap, part * 2 * J, [[J, 2], [0, 8], [1, J]]),
                    )
                return k_bf

            def emit_qt(t, k_bf, full_act=False):
                # transposes: qt_ps[p=d%128, (part, dc, c, m)]
                qt_ps = qtps.tile([128, 1024], BF16, tag="qt", name="qt_ps")
                for part in range(2):
                    for c in range(NK):
                        for dc in range(2):
                            nc.tensor.matmul(
                                qt_ps[:, part * 512 + dc * 256 + c * 128:
                                      part * 512 + dc * 256 + c * 128 + 128],
                                k_bf[:, part * 512 + c * 256 + dc * 128:
                                     part * 512 + c * 256 + dc * 128 + 128],
                                identb,
                                is_transpose=True,
                            )
                # escape -> fp8 (no scale; group weight applied at A escape)
                qt_sb = qtp.tile([128, 1024], E4, tag="qtsb", name="qt_sb")
                if full_act:
                    nc.scalar.activation(qt_sb, qt_ps, ACTF.Copy)
                else:
                    nc.scalar.activation(qt_sb[:, 0:768], qt_ps[:, 0:768],
                                         ACTF.Copy)
                    nc.vector.tensor_copy(qt_sb[:, 768:1024],
                                          qt_ps[:, 768:1024])
                return ("f8", qt_sb)

            def emit_qt_dma(t, k_bf):
                # DMA xbar transpose, no compute engine: k_bf[m, (part,c,dh,
                # dl)] -> qt_bf[dl, (part, c, dh, m)] (g = (part,c,dh) blocks)
                qt_bf = qtbp.tile([128, 1024], BF16, tag="qtbf", name="qt_bf")
                nc.sync.dma_start_transpose(
                    qt_bf[:].rearrange("p (g m) -> p g m", g=8),
                    k_bf[:],
                )
                return ("bf", qt_bf)

            def emit_A(t, qt):
                kind, qt_sb = qt
                acc = A_g0 if t < NG0 else A_g1
                first = t in (0, NG0)
                last = t in (NG0 - 1, NT - 1)
                if kind == "f8":
                    # qt_sb fp8 [128, (part, dc, c*128+m)]: DR, K=256/instr
                    for part in range(2):
                        for mc in range(2):
                            nc.tensor.matmul(
                                acc[:, mc * 256:mc * 256 + 256],
                                _ap(qt_sb[:], part * 512 + mc * 128,
                                    [[256, 2], [1, 128]]),
                                _ap(qt_sb[:], part * 512,
                                    [[256, 2], [1, 256]]),
                                start=(first and part == 0 and mc == 0),
                                stop=(last and part == 1 and mc == 1),
                                perf_mode=DR,
                            )
                    return
                # qt_sb bf16 [128, (part, c, dh, m)]: K=128/instr
                n8 = 0
                for part in range(2):
                    for mc in range(2):
                        for dh in range(2):
                            n8 += 1
                            nc.tensor.matmul(
                                acc[:, mc * 256:mc * 256 + 256],
                                qt_sb[:, part * 512 + mc * 256 + dh * 128:
                                      part * 512 + mc * 256 + dh * 128 + 128],
                                _ap(qt_sb[:], part * 512 + dh * 128,
                                    [[256, 2], [1, 128]]),
                                start=(first and n8 == 1),
                                stop=(last and n8 == 8),
                            )

            def emit_Aesc(a_ps, a_sb, w_col, eng):
                # A_sb[p, (mc, m')] = w_g * A[mc*128+p, m'] (exact f32 w)
                if eng == "dve":
                    nc.vector.tensor_scalar_mul(a_sb, a_ps, w_col)
                else:
                    nc.scalar.activation(a_sb, a_ps, ACTF.Copy, scale=w_col)

            def emit_AVmm(a_sb, start, stop):
                # symmetric A: the same tile serves as lhsT for both chunks
                for oc in range(2):
                    for kc in range(2):
                        nc.tensor.matmul(
                            out_ps[:, oc * 256:oc * 256 + 256],
                            a_sb[:, kc * 256 + oc * 128:
                                 kc * 256 + oc * 128 + 128],
                            x_sb[:, kc * 256:(kc + 1) * 256],
                            start=(start and oc == 0 and kc == 0),
                            stop=(stop and oc == 1 and kc == 1),
                        )

            DMA_TERMS = (0, 1, 2)

            sc_tiles = {}
            sc_tiles.update(emit_angle_pair([0, 1], fast=True))
            sc_tiles.update(emit_angle_pair([2], fast=True))
            sc_tiles.update(emit_angle_pair([3, 4], fast=False))

            A_sb0 = cp.tile([128, 2 * D], BF16)
            A_sb1 = cp.tile([128, 2 * D], BF16)

            qts = {}
            for t in range(NT):
                k_bf = emit_muls(t, sc_tiles.pop(t))
                if t == 0:
                    sc_tiles.update(emit_angle_pair([5, 6], fast=False))
                # PE keep-alive: one transpose per k_bf keeps the p-state
                # ramp warm across the window where PE waits for dma-qt.
                warm_ps = warmp.tile([128, 128], BF16, tag="warm",
                                     name="warm_ps")
                nc.tensor.matmul(warm_ps[:, 0:128], k_bf[:, 0:128], identb,
                                 is_transpose=True)
                if t in DMA_TERMS:
                    qts[t] = emit_qt_dma(t, k_bf)
                else:
                    qts[t] = emit_qt(t, k_bf)
            # group pair-weights [128, 2] (needed only by the A escapes)
            w_bc = cp.tile([128, 2], F32)
            nc.gpsimd.partition_broadcast(w_bc, aux_sb[0:1, C_W:C_W + 2])
            for t in range(NT):
                emit_A(t, qts.pop(t))
            emit_Aesc(A_g0, A_sb0, w_bc[:, 0:1], "dve")
            emit_Aesc(A_g1, A_sb1, w_bc[:, 1:2], "act")
            emit_AVmm(A_sb0, start=True, stop=False)
            emit_AVmm(A_sb1, start=False, stop=True)

            # ---------------- tail: escape + store (per chunk) ----------
            out_sb = cp.tile([128, NK * D], BF16)
            nc.scalar.activation(out_sb[:, 0:256], out_ps[:, 0:256], ACTF.Copy)
            nc.vector.tensor_copy(out_sb[:, 256:512], out_ps[:, 256:512])
            nc.sync.dma_start(out=OUT[:, 0:256], in_=out_sb[:, 0:256])
            nc.scalar.dma_start(out=OUT[:, 256:512], in_=out_sb[:, 256:512])

    nc.finalize()
    return nc


_PROGRAM_CACHE = {}


def _get_program():
    if "p" not in _PROGRAM_CACHE:
        _PROGRAM_CACHE["p"] = _build_program()
    return _PROGRAM_CACHE["p"]


def _find_pairs(grid_u, grid_w):
    """Antipodal pairs with equal weights; assert full pairing."""
    G = grid_u.shape[0]
    used = [False] * G
    pairs = []
    for i in range(G):
        if used[i]:
            continue
        partner = -1
        for j in range(i + 1, G):
            if used[j]:
                continue
            if (np.allclose(grid_u[j], -grid_u[i], rtol=1e-6, atol=1e-7)
                    and abs(float(grid_w[j]) - float(grid_w[i])) <= 1e-7):
                partner = j
                break
        used[i] = True
        assert partner >= 0, "unpaired grid direction"
        used[partner] = True
        pairs.append((i, partner))
    return pairs


def _prepare(inputs, positions, batch_segments, graph_mask, frequencies, grid_u,
             grid_w):
    n, p, s, f = inputs.shape
    d = p * s * f
    b = graph_mask.shape[0]
    G = grid_u.shape[0]
    Jn = frequencies.shape[0]
    assert (n, d, b, G, Jn) == (2048, 256, 8, 14, 32), (n, d, b, G, Jn)

    x = np.asarray(inputs, np.float32).reshape(n, d)
    pos = np.asarray(positions, np.float32)
    seg = np.asarray(batch_segments)
    gmask = np.asarray(graph_mask)
    gu = np.asarray(grid_u, np.float32)
    gw = np.asarray(grid_w, np.float32)

    idxs = [np.nonzero(seg == c)[0] for c in range(b)]
    assert max(len(ix) for ix in idxs) <= M

    pairs = _find_pairs(gu, gw)
    assert len(pairs) == NT
    # group pairs by weight: exactly two distinct values, sizes (NG0, NT-NG0)
    pw = np.array([gw[i] + gw[j] for i, j in pairs], np.float32)
    vals = np.unique(pw)
    assert len(vals) == 2, vals
    g0 = [k for k in range(NT) if pw[k] == vals[0]]
    g1 = [k for k in range(NT) if pw[k] == vals[1]]
    if len(g0) != NG0:
        g0, g1 = g1, g0
    assert len(g0) == NG0 and len(g1) == NT - NG0, (g0, g1)
    order = g0 + g1
    pairs = [pairs[k] for k in order]
    wgrp = np.array([pw[order[0]], pw[order[NG0]]], np.float32)
    reps = [i for i, _ in pairs]

    in_maps = []
    for c in range(b):
        ix = idxs[c]
        pad = np.zeros(M, np.int64)
        pad[:len(ix)] = ix
        mask = np.zeros(M, np.float32)
        mask[:len(ix)] = gmask[seg[ix]].astype(np.float32)

        xm = x[pad] * mask[:, None]
        # d-permute (s, f=2j+r) -> (s, r, j)
        xp = xm.reshape(M, 4, J, 2).transpose(0, 1, 3, 2).reshape(M, d)
        x_prep = np.ascontiguousarray(
            xp.reshape(NK, 128, d).transpose(1, 0, 2).reshape(128, NK * d))
        aux = np.zeros((128, W_AUX), np.float32)
        aux[0:3, C_POST:C_POST + M] = pos[pad].T
        aux[0:3, C_UT:C_UT + NT] = gu[reps].T
        aux[0, C_FRQ:C_FRQ + J] = (np.asarray(frequencies, np.float32)
                                   / np.float32(TWO_PI))
        aux[0, C_W:C_W + 2] = wgrp
        in_maps.append(dict(x=x_prep.astype(ml_dtypes.bfloat16), aux=aux))

    meta = dict(n=n, p=p, s=s, f=f, d=d, b=b, idxs=idxs, pairs=pairs)
    return in_maps, meta


def _gather(results, meta, dtype):
    n, d = meta["n"], meta["d"]
    out = np.zeros((n, d), np.float32)
    for c, ix in enumerate(meta["idxs"]):
        o = np.asarray(results[c]["out"]).astype(np.float32)
        o_nodes = o.reshape(128, NK, d).transpose(1, 0, 2).reshape(M, d)
        # un-permute (s, r, j) -> (s, f=2j+r)
        o_un = o_nodes.reshape(M, 4, 2, J).transpose(0, 1, 3, 2).reshape(M, d)
        out[ix] = o_un[:len(ix)]
    return out.reshape(n, meta["p"], meta["s"], meta["f"]).astype(dtype)


def _run(inputs, positions, batch_segments, graph_mask, frequencies, grid_u,
         grid_w, trace=False):
    in_maps, meta = _prepare(inputs, positions, batch_segments, graph_mask,
                             frequencies, grid_u, grid_w)
    nc = _get_program()
    res = run_bass_kernel_spmd(
        nc, in_maps, core_ids=list(range(N_CORES)), trace=trace
    )
    out = _gather(res.results, meta, np.asarray(inputs).dtype)
    return out, res


def kernel(inputs, positions, batch_segments, graph_mask, frequencies, grid_u,
           grid_w):
    out, _ = _run(inputs, positions, batch_segments, graph_mask, frequencies,
                  grid_u, grid_w)
    return out
